# revision 1
# baseline (speedup 1.0000x reference)
"""RMT memory kernel for 8 Trainium2 NeuronCores.

Core c = (batch b=c//4, token-quarter t=c%4); 256 tokens/segment/core.
Read-attention, gates, h token-local. Write-attention numerators/denoms
AllReduced per 4-core batch group (AR#1). Memory update replicated
post-AR; next-segment k_r/v_r projections K-sharded over the group and
summed in AR#2, which also redistributes the full memory state.

Everything on device is D-major ("transposed"); host pre-transposes
inputs and re-transposes outputs. All matmuls bf16 (host-cast weights);
the only precision-critical op, h = seg + gate*o, adds seg in fp32.
"""

import sys

sys.path.insert(0, "/opt/trn_rl_repo")

import numpy as np
import ml_dtypes

D = 2560
H = 8
HD = 320
MEM = 128
NSEG = 4
TOK = 256
NT = D // 128       # 20
SL = 640            # per-core D slice
GROUPS = [[0, 1, 2, 3], [4, 5, 6, 7]]
SCALE = 1.0 / float(np.sqrt(HD))
BF16 = ml_dtypes.bfloat16

_CACHE = {}


def _pieces(start, size):
    """Split [start, start+size) at 128-boundaries."""
    out, p, end = [], start, start + size
    while p < end:
        nxt = min(end, (p // 128 + 1) * 128)
        out.append((p, nxt - p))
        p = nxt
    return out


def _build(debug=False):
    import concourse.bass as bass
    import concourse.bacc as bacc
    import concourse.mybir as mybir
    import concourse.tile as tile

    f32 = mybir.dt.float32
    bf16 = mybir.dt.bfloat16
    AF = mybir.ActivationFunctionType

    nc = bacc.Bacc("TRN2", target_bir_lowering=False, debug=False, num_devices=8)

    xt32 = nc.dram_tensor("xt32", [D, NSEG * TOK], f32, kind="ExternalInput")
    xtb = nc.dram_tensor("xtb", [D, NSEG * TOK], bf16, kind="ExternalInput")
    wqr = nc.dram_tensor("wqr", [D, D], bf16, kind="ExternalInput")
    wgr = nc.dram_tensor("wgr", [D, D], bf16, kind="ExternalInput")
    wqw = nc.dram_tensor("wqw", [D, D], bf16, kind="ExternalInput")
    wor = nc.dram_tensor("wor", [D, D], bf16, kind="ExternalInput")
    wkw = nc.dram_tensor("wkw", [D, D], bf16, kind="ExternalInput")
    wvw = nc.dram_tensor("wvw", [D, D], bf16, kind="ExternalInput")
    wow = nc.dram_tensor("wow", [D, D], bf16, kind="ExternalInput")
    wgw_sl = nc.dram_tensor("wgw_sl", [2 * D, SL], bf16, kind="ExternalInput")
    wkr_sl = nc.dram_tensor("wkr_sl", [SL, D], bf16, kind="ExternalInput")
    wvr_sl = nc.dram_tensor("wvr_sl", [SL, D], bf16, kind="ExternalInput")
    bgr = nc.dram_tensor("bgr", [D, 1], f32, kind="ExternalInput")
    bgw_sl = nc.dram_tensor("bgw_sl", [SL, 1], f32, kind="ExternalInput")
    mqt = nc.dram_tensor("mqt", [D, MEM], bf16, kind="ExternalInput")
    m0t = nc.dram_tensor("m0t", [D, MEM], bf16, kind="ExternalInput")

    ht = nc.dram_tensor("ht", [D, NSEG * TOK], f32, kind="ExternalOutput")
    dbg = nc.dram_tensor("dbg", [128, 53504], bf16, kind="ExternalOutput") \
        if debug else None

    with tile.TileContext(nc) as tc:
        with (
            tc.tile_pool(name="dram", bufs=1, space="DRAM") as dram,
            tc.tile_pool(name="res", bufs=1) as res,
            tc.tile_pool(name="wrk", bufs=1) as wrk,
            tc.tile_pool(name="stream", bufs=1) as stream,
            tc.tile_pool(name="ps", bufs=8, space="PSUM") as psp,
        ):
            # ---------------- DRAM scratch ----------------
            qr_sp = dram.tile([128, NT * NSEG * TOK], bf16)  # seg-major
            gt_sp = dram.tile([128, NT * NSEG * TOK], bf16)
            b1_in = dram.tile([129, NT * 128], bf16)
            b1_out = dram.tile([129, NT * 128], bf16)
            b2_in = dram.tile([384, NT * 128], bf16)
            b2_out = dram.tile([384, NT * 128], bf16)

            def psum(n=512):
                t = psp.tile([128, 512], f32, tag="ps", bufs=8)
                return t[:, :n] if n != 512 else t[:]

            _dbg_off = [0]

            def dump(ap, cols):
                if dbg is None:
                    return
                o = _dbg_off[0]
                nc.gpsimd.dma_start(dbg[:, o:o + cols], ap)
                _dbg_off[0] = o + cols

            # ---------------- residents ----------------
            xtb_sb = res.tile([128, NT * NSEG * TOK], bf16)
            qwT = res.tile([128, NT * 128], bf16)
            mem = res.tile([128, NT * 128], bf16)
            krT = res.tile([128, NT * 128], bf16)
            vr = res.tile([128, NT * 128], bf16)
            ones_bf = res.tile([128, 1], bf16)
            nc.vector.memset(ones_bf[:], 1.0)

            for k in range(NT):
                nc.gpsimd.dma_start(
                    xtb_sb[:, k * NSEG * TOK:(k + 1) * NSEG * TOK],
                    xtb[k * 128:(k + 1) * 128, :])
                nc.gpsimd.dma_start(mem[:, k * 128:(k + 1) * 128],
                                    m0t[k * 128:(k + 1) * 128, :])

            pid = nc.vector.partition_id()
            toff = nc.snap((pid % 4) * SL, donate=False, min_val=0,
                           max_val=3 * SL)

            # ---------------- helpers ----------------
            def gemm(w_dram, rhs_get, n, evac, kt=NT, mt=NT, mg=4, tag="w",
                     eng=None):
                """out^T[m, :n] += sum_k w[k, m-block].T @ rhs_get(k).

                Streams one [128, mg*128] weight tile per k (sequential use,
                bufs=3 prefetch); mg psum tiles stay resident across the
                k-accumulation."""
                eng = eng or nc.sync
                for m0 in range(0, mt, mg):
                    g = min(mg, mt - m0)
                    pss = [psum(n) for _ in range(g)]
                    for k in range(kt):
                        wt = stream.tile([128, g * 128], bf16, tag=tag, bufs=3)
                        eng.dma_start(
                            wt[:], w_dram[k * 128:(k + 1) * 128,
                                          m0 * 128:(m0 + g) * 128])
                        r = rhs_get(k)
                        for mi in range(g):
                            nc.tensor.matmul(
                                pss[mi], wt[:, mi * 128:(mi + 1) * 128], r,
                                start=(k == 0), stop=(k == kt - 1))
                    for mi in range(g):
                        evac(m0 + mi, pss[mi])

            # ---------------- q_w^T (one-time) ----------------
            def mqt_rhs(k):
                t = stream.tile([128, 128], bf16, tag="rstream", bufs=3)
                nc.gpsimd.dma_start(t[:], mqt[k * 128:(k + 1) * 128, :])
                return t[:]
            gemm(wqw, mqt_rhs, 128,
                 lambda m, ps: nc.scalar.copy(qwT[:, m * 128:(m + 1) * 128], ps),
                 mg=4, tag="w")

            # ---------------- k/v partials + AR#2 ----------------
            def kv_partials_and_ar2(memsl_src, memstage_or_none):
                """memsl_src: [128, 5*128] slice rows of (updated) mem^T."""
                pk = wrk.tile([128, NT * 128], bf16, tag="pkpv", bufs=2)
                pv = wrk.tile([128, NT * 128], bf16, tag="pkpv", bufs=2)
                gemm(wkr_sl,
                     lambda kk: memsl_src[:, kk * 128:(kk + 1) * 128], 128,
                     lambda m, ps: nc.scalar.copy(
                         pk[:, m * 128:(m + 1) * 128], ps),
                     kt=5, mt=NT, mg=4, tag="w")
                for ch in range(5):
                    ps = psum(512)
                    for kk in range(5):
                        wt = stream.tile([128, 512], bf16, tag="w", bufs=3)
                        nc.sync.dma_start(
                            wt[:], wvr_sl[kk * 128:(kk + 1) * 128,
                                          ch * 512:(ch + 1) * 512])
                        nc.tensor.matmul(
                            ps, memsl_src[:, kk * 128:(kk + 1) * 128],
                            wt[:], start=(kk == 0), stop=(kk == 4))
                    nc.scalar.copy(pv[:, ch * 512:(ch + 1) * 512], ps)
                nc.gpsimd.dma_start(b2_in[0:128, :], pk[:])
                nc.gpsimd.dma_start(b2_in[128:256, :], pv[:])
                if memstage_or_none is not None:
                    nc.gpsimd.dma_start(b2_in[256:384, :], memstage_or_none[:])
                nc.gpsimd.collective_compute(
                    "AllReduce", mybir.AluOpType.add,
                    ins=[b2_in.opt()], outs=[b2_out.opt()],
                    replica_groups=GROUPS)
                nc.gpsimd.dma_start(krT[:], b2_out[0:128, :])
                nc.gpsimd.dma_start(vr[:], b2_out[128:256, :])
                if memstage_or_none is not None:
                    nc.gpsimd.dma_start(mem[:], b2_out[256:384, :])

            # seg-0 bootstrap: k_r/v_r from initial memory
            memsl0 = wrk.tile([128, 5 * 128], bf16, tag="memsl")
            nc.vector.tensor_copy(memsl0[:], mem[:, bass.ds(toff, SL)])
            kv_partials_and_ar2(memsl0, None)

            # ---------------- phase-1: q_r^T, gate^T ----------------
            def phase1(w_dram, sp, bias_dram, tag):
                def evac(m, chunk, ps):
                    if bias_dram is None:
                        tmp = wrk.tile([128, 512], bf16, tag="p1ev", bufs=2)
                        nc.scalar.copy(tmp[:], ps)
                    else:
                        bt = wrk.tile([128, 1], f32, tag="p1b", bufs=2)
                        nc.scalar.dma_start(
                            bt[:], bias_dram[m * 128:(m + 1) * 128, :])
                        tmp = wrk.tile([128, 512], bf16, tag="p1ev", bufs=2)
                        nc.scalar.activation(tmp[:], ps, AF.Sigmoid, bias=bt[:])
                    for sh in range(2):  # two segments per 512 chunk
                        s = chunk * 2 + sh
                        dst_col = s * NT * TOK + m * TOK
                        nc.scalar.dma_start(
                            sp[:, dst_col:dst_col + TOK],
                            tmp[:, sh * TOK:(sh + 1) * TOK])
                for m0 in range(0, NT, 2):
                    pss = [[psum(512) for _ in range(2)] for _ in range(2)]
                    for k in range(NT):
                        wt = stream.tile([128, 256], bf16, tag=tag, bufs=3)
                        nc.scalar.dma_start(
                            wt[:], w_dram[k * 128:(k + 1) * 128,
                                          m0 * 128:(m0 + 2) * 128])
                        for mi in range(2):
                            for chunk in range(2):
                                nc.tensor.matmul(
                                    pss[mi][chunk],
                                    wt[:, mi * 128:(mi + 1) * 128],
                                    xtb_sb[:, k * NSEG * TOK + chunk * 512:
                                           k * NSEG * TOK + (chunk + 1) * 512],
                                    start=(k == 0), stop=(k == NT - 1))
                    for mi in range(2):
                        for chunk in range(2):
                            evac(m0 + mi, chunk, pss[mi][chunk])

            phase1(wqr, qr_sp, None, "p1q")
            phase1(wgr, gt_sp, bgr, "p1g")

            # ---------------- segment chain ----------------
            for s in range(NSEG):
                # -- reload q_r^T / gate^T for this segment --
                qrT_sb = wrk.tile([128, NT * TOK], bf16, tag="qrT")
                gtT_sb = wrk.tile([128, NT * TOK], bf16, tag="gtT")
                nc.scalar.dma_start(
                    qrT_sb[:], qr_sp[:, s * NT * TOK:(s + 1) * NT * TOK])
                nc.scalar.dma_start(
                    gtT_sb[:], gt_sp[:, s * NT * TOK:(s + 1) * NT * TOK])

                # -- read attention --
                E_sb = wrk.tile([128, H * TOK], bf16, tag="E")
                for h in range(H):
                    ps = psum(TOK)
                    pcs = _pieces(h * HD, HD)
                    for i, (off, sz) in enumerate(pcs):
                        g, o = off // 128, off % 128
                        nc.tensor.matmul(
                            ps, krT[o:o + sz, g * 128:(g + 1) * 128],
                            qrT_sb[o:o + sz, g * TOK:(g + 1) * TOK],
                            start=(i == 0), stop=(i == len(pcs) - 1))
                    nc.scalar.activation(E_sb[:, h * TOK:(h + 1) * TOK], ps,
                                         AF.Exp, scale=SCALE)
                # read denominators: [1, 256] per head, 4 heads per psum tile
                dpack = wrk.tile([128, H * TOK], f32, tag="dpack")
                dens = dpack[0:1, :]
                for hp in range(H // 2):
                    ps = psum(512)
                    for hh in range(2):
                        h = hp * 2 + hh
                        nc.tensor.matmul(
                            ps[0:1, hh * TOK:(hh + 1) * TOK], ones_bf[:],
                            E_sb[:, h * TOK:(h + 1) * TOK],
                            start=True, stop=True)
                    nc.vector.tensor_copy(
                        dens[:, hp * 512:(hp + 1) * 512], ps[0:1, :])
                recip = wrk.tile([1, H * TOK], f32, tag="recip")
                nc.vector.reciprocal(recip[:], dens[:])
                rrep = {}
                for h in range(H):
                    rrep[h] = wrk.tile([128, TOK], f32, tag="rrep", bufs=2, name="rrep_h")
                    nc.gpsimd.partition_broadcast(
                        rrep[h][:], recip[:, h * TOK:(h + 1) * TOK])

                # ctx (normalized at evacuation)
                ctx_sb = wrk.tile([128, NT * TOK], bf16, tag="ctx")
                ctx_ps = {}
                for h in range(H):
                    for off, sz in _pieces(h * HD, HD):
                        g, o = off // 128, off % 128
                        if g not in ctx_ps:
                            ctx_ps[g] = (psum(TOK), [])
                        ps, evl = ctx_ps[g]
                        nc.tensor.matmul(
                            ps[o:o + sz, :], vr[:, off:off + sz],
                            E_sb[:, h * TOK:(h + 1) * TOK],
                            start=True, stop=True)
                        evl.append((h, o, sz))
                        if o + sz == 128 or (h == H - 1 and off + sz == D):
                            for (hh, oo, ss) in evl:
                                nc.vector.tensor_mul(
                                    ctx_sb[oo:oo + ss, g * TOK:(g + 1) * TOK],
                                    ps[oo:oo + ss, :],
                                    rrep[hh][oo:oo + ss, :])
                            ctx_ps[g] = None

                if s == 0:
                    dump(krT[:], NT * 128)
                    dump(vr[:], NT * 128)
                    dump(qrT_sb[:], NT * TOK)
                    dump(gtT_sb[:], NT * TOK)
                    dump(E_sb[:], H * TOK)
                    dump(ctx_sb[:], NT * TOK)

                # -- o^T = wo_r.T @ ctx ; h = seg + gate*o --
                hbf = wrk.tile([128, NT * TOK], bf16, tag="hbf")

                def evac_o(m, ps):
                    seg32 = wrk.tile([128, TOK], f32, tag="seg32", bufs=2)
                    nc.scalar.dma_start(
                        seg32[:], xt32[m * 128:(m + 1) * 128,
                                       s * TOK:(s + 1) * TOK])
                    tmp = wrk.tile([128, TOK], f32, tag="otmp", bufs=2)
                    nc.vector.tensor_mul(
                        tmp[:], ps, gtT_sb[:, m * TOK:(m + 1) * TOK])
                    h32 = wrk.tile([128, TOK], f32, tag="h32", bufs=2)
                    nc.vector.tensor_add(h32[:], tmp[:], seg32[:])
                    nc.scalar.dma_start(
                        ht[m * 128:(m + 1) * 128, s * TOK:(s + 1) * TOK],
                        h32[:])
                    nc.vector.tensor_copy(hbf[:, m * TOK:(m + 1) * TOK],
                                          h32[:])

                gemm(wor, lambda k: ctx_sb[:, k * TOK:(k + 1) * TOK], TOK,
                     evac_o, mg=4, tag="w")

                # -- k_w^T, v_w --
                kwT = wrk.tile([128, NT * TOK], bf16, tag="kwT")
                gemm(wkw, lambda k: hbf[:, k * TOK:(k + 1) * TOK], TOK,
                     lambda m, ps: nc.scalar.copy(
                         kwT[:, m * TOK:(m + 1) * TOK], ps),
                     mg=4, tag="w", eng=nc.scalar)
                vw = wrk.tile([128, 2 * D], bf16, tag="vw")
                for ch in range(5):
                    pss = [psum(512), psum(512)]
                    for k in range(NT):
                        wt = stream.tile([128, 512], bf16, tag="w", bufs=3)
                        nc.scalar.dma_start(
                            wt[:], wvw[k * 128:(k + 1) * 128,
                                       ch * 512:(ch + 1) * 512])
                        for tt in range(2):
                            nc.tensor.matmul(
                                pss[tt],
                                hbf[:, k * TOK + tt * 128:k * TOK + tt * 128 + 128],
                                wt[:], start=(k == 0), stop=(k == NT - 1))
                    for tt in range(2):
                        nc.scalar.copy(
                            vw[:, tt * D + ch * 512:tt * D + (ch + 1) * 512],
                            pss[tt])

                if s == 0:
                    dump(hbf[:], NT * TOK)
                    dump(kwT[:], NT * TOK)
                    dump(vw[:], 2 * D)

                # -- write attention --
                ewT = wrk.tile([128, H * 2 * 128], bf16, tag="ewT")
                for h in range(H):
                    for tt in range(2):
                        ps = psum(128)
                        pcs = _pieces(h * HD, HD)
                        for i, (off, sz) in enumerate(pcs):
                            g, o = off // 128, off % 128
                            nc.tensor.matmul(
                                ps,
                                kwT[o:o + sz,
                                    g * TOK + tt * 128:g * TOK + tt * 128 + 128],
                                qwT[o:o + sz, g * 128:(g + 1) * 128],
                                start=(i == 0), stop=(i == len(pcs) - 1))
                        nc.scalar.activation(
                            ewT[:, (h * 2 + tt) * 128:(h * 2 + tt + 1) * 128],
                            ps, AF.Exp, scale=SCALE)
                densw = dpack[64:65, 0:H * 128]
                for hp in range(H // 4):
                    ps = psum(512)
                    for hh in range(4):
                        h = hp * 4 + hh
                        for tt in range(2):
                            nc.tensor.matmul(
                                ps[0:1, hh * 128:(hh + 1) * 128], ones_bf[:],
                                ewT[:, (h * 2 + tt) * 128:(h * 2 + tt + 1) * 128],
                                start=(tt == 0), stop=(tt == 1))
                    nc.vector.tensor_copy(
                        densw[:, hp * 512:(hp + 1) * 512], ps[0:1, :])
                dbpack = wrk.tile([64, H * 128], bf16, tag="dbpack")
                densw_bf = dbpack[0:1, :]
                nc.vector.tensor_copy(densw_bf[:], densw[:])

                attn = wrk.tile([128, NT * 128], bf16, tag="attn", bufs=2)
                att_ps = {}
                for h in range(H):
                    for off, sz in _pieces(h * HD, HD):
                        g, o = off // 128, off % 128
                        if g not in att_ps:
                            att_ps[g] = (psum(128), [])
                        ps, evl = att_ps[g]
                        for tt in range(2):
                            nc.tensor.matmul(
                                ps[o:o + sz, :],
                                vw[:, tt * D + off:tt * D + off + sz],
                                ewT[:, (h * 2 + tt) * 128:(h * 2 + tt + 1) * 128],
                                start=(tt == 0), stop=(tt == 1))
                        evl.append((h, o, sz))
                        if o + sz == 128 or (h == H - 1 and off + sz == D):
                            for (hh, oo, ss) in evl:
                                nc.scalar.copy(
                                    attn[oo:oo + ss, g * 128:(g + 1) * 128],
                                    ps[oo:oo + ss, :])
                            att_ps[g] = None

                if s == 0:
                    dump(ewT[:], H * 2 * 128)
                    dump(attn[:], NT * 128)

                # -- AR#1 --
                nc.gpsimd.dma_start(b1_in[0:128, :], attn[:])
                nc.gpsimd.dma_start(b1_in[128:129, 0:H * 128], densw_bf[:])
                nc.gpsimd.collective_compute(
                    "AllReduce", mybir.AluOpType.add,
                    ins=[b1_in.opt()], outs=[b1_out.opt()],
                    replica_groups=GROUPS)
                attnS = wrk.tile([128, NT * 128], bf16, tag="attn", bufs=2)
                denswS = dbpack[32:33, :]
                nc.gpsimd.dma_start(attnS[:], b1_out[0:128, :])
                nc.gpsimd.dma_start(denswS[:], b1_out[128:129, 0:H * 128])
                denswS32 = dpack[96:97, 0:H * 128]
                nc.vector.tensor_copy(denswS32[:], denswS[:])
                recw = wrk.tile([1, H * 128], f32, tag="recw")
                nc.vector.reciprocal(recw[:], denswS32[:])
                rwrep = {}
                for h in range(H):
                    rwrep[h] = wrk.tile([128, 128], f32, tag="rwrep", bufs=2, name="rwrep_h")
                    nc.gpsimd.partition_broadcast(
                        rwrep[h][:], recw[:, h * 128:(h + 1) * 128])
                attnN = wrk.tile([128, NT * 128], bf16, tag="attn", bufs=2)
                for h in range(H):
                    for off, sz in _pieces(h * HD, HD):
                        g, o = off // 128, off % 128
                        nc.vector.tensor_mul(
                            attnN[o:o + sz, g * 128:(g + 1) * 128],
                            attnS[o:o + sz, g * 128:(g + 1) * 128],
                            rwrep[h][o:o + sz, :])

                if s == 0:
                    dump(attnN[:], NT * 128)

                # -- new_mem^T (full, replicated) --
                new_sb = wrk.tile([128, NT * 128], bf16, tag="new")
                gemm(wow, lambda k: attnN[:, k * 128:(k + 1) * 128], 128,
                     lambda m, ps: nc.scalar.copy(
                         new_sb[:, m * 128:(m + 1) * 128], ps),
                     mg=4, tag="w", eng=nc.scalar)

                # -- gate (slice-local) + memory update --
                g_ps = [psum(128) for _ in range(5)]
                for k in range(2 * NT):
                    wt = stream.tile([128, SL], bf16, tag="wgw", bufs=3)
                    nc.sync.dma_start(
                        wt[:], wgw_sl[k * 128:(k + 1) * 128, :])
                    rhs = (mem[:, k * 128:(k + 1) * 128] if k < NT
                           else new_sb[:, (k - NT) * 128:(k - NT + 1) * 128])
                    for m in range(5):
                        nc.tensor.matmul(
                            g_ps[m], wt[:, m * 128:(m + 1) * 128], rhs,
                            start=(k == 0), stop=(k == 2 * NT - 1))
                gsl = wrk.tile([128, 5 * 128], f32, tag="gsl")
                for m in range(5):
                    bt = wrk.tile([128, 1], f32, tag="bgw_t", bufs=2)
                    nc.scalar.dma_start(bt[:], bgw_sl[m * 128:(m + 1) * 128, :])
                    nc.scalar.activation(gsl[:, m * 128:(m + 1) * 128],
                                         g_ps[m], AF.Sigmoid, bias=bt[:])

                new_sl = wrk.tile([128, 5 * 128], bf16, tag="new_sl")
                nc.vector.tensor_copy(new_sl[:], new_sb[:, bass.ds(toff, SL)])
                memsl = wrk.tile([128, 5 * 128], bf16, tag="memsl")
                nc.vector.tensor_copy(memsl[:], mem[:, bass.ds(toff, SL)])
                dlt = wrk.tile([128, 5 * 128], f32, tag="dlt")
                nc.vector.tensor_sub(dlt[:], new_sl[:], memsl[:])
                nc.vector.tensor_mul(dlt[:], gsl[:], dlt[:])
                memsl_n = wrk.tile([128, 5 * 128], bf16, tag="memsl_n")
                nc.vector.tensor_add(memsl_n[:], memsl[:], dlt[:])

                if s == 0 and dbg is not None:
                    dump(new_sb[:], NT * 128)
                    gslb = wrk.tile([128, 5 * 128], bf16, tag="gslb")
                    nc.vector.tensor_copy(gslb[:], gsl[:])
                    dump(gslb[:], 5 * 128)
                    dump(memsl_n[:], 5 * 128)
                if s < NSEG - 1:
                    memstage = wrk.tile([128, NT * 128], bf16, tag="attn", bufs=2)
                    nc.vector.memset(memstage[:], 0.0)
                    nc.vector.tensor_copy(
                        memstage[:, bass.ds(toff, SL)], memsl_n[:])
                    kv_partials_and_ar2(memsl_n, memstage)

    nc.compile()
    return nc


def _prep_inputs(inputs):
    hs = np.asarray(inputs["hidden_states"], np.float32)
    Bsz = hs.shape[0]

    def bf(x):
        return np.ascontiguousarray(np.asarray(x, np.float32).astype(BF16))

    shared = {
        "wqr": bf(inputs["wq_r"]), "wgr": bf(inputs["wg_r"]),
        "wqw": bf(inputs["wq_w"]), "wor": bf(inputs["wo_r"]),
        "wkw": bf(inputs["wk_w"]), "wvw": bf(inputs["wv_w"]),
        "wow": bf(inputs["wo_w"]),
        "bgr": np.ascontiguousarray(
            np.asarray(inputs["bg_r"], np.float32)[:, None]),
        "mqt": bf(np.asarray(inputs["write_queries"], np.float32)[0].T),
        "m0t": bf(np.asarray(inputs["initial_memory"], np.float32)[0].T),
    }
    wgw = np.asarray(inputs["wg_w"], np.float32)
    wkr = np.asarray(inputs["wk_r"], np.float32)
    wvr = np.asarray(inputs["wv_r"], np.float32)
    bgw = np.asarray(inputs["bg_w"], np.float32)

    in_maps = []
    for c in range(8):
        b, t = c // 4, c % 4
        cols = np.concatenate(
            [np.arange(s * 1024 + t * TOK, s * 1024 + (t + 1) * TOK)
             for s in range(NSEG)])
        xt = np.ascontiguousarray(hs[b, cols, :].T)  # [D, 1024]
        im = dict(shared)
        im["xt32"] = xt
        im["xtb"] = np.ascontiguousarray(xt.astype(BF16))
        im["wgw_sl"] = np.ascontiguousarray(
            wgw[:, t * SL:(t + 1) * SL].astype(BF16))
        im["wkr_sl"] = np.ascontiguousarray(
            wkr[t * SL:(t + 1) * SL, :].astype(BF16))
        im["wvr_sl"] = np.ascontiguousarray(
            wvr[t * SL:(t + 1) * SL, :].astype(BF16))
        im["bgw_sl"] = np.ascontiguousarray(bgw[t * SL:(t + 1) * SL, None])
        in_maps.append(im)
    return in_maps


def _run(inputs, trace=False, debug=False):
    from concourse.bass_utils import run_bass_kernel_spmd
    key = ("nc", debug)
    if key not in _CACHE:
        _CACHE[key] = _build(debug=debug)
    in_maps = _prep_inputs(inputs)
    res = run_bass_kernel_spmd(_CACHE[key], in_maps, list(range(8)),
                               trace=trace)
    hs = np.asarray(inputs["hidden_states"])
    out = np.empty((hs.shape[0], NSEG * 1024, D), np.float32)
    for c in range(8):
        b, t = c // 4, c % 4
        htc = res.results[c]["ht"]  # [D, NSEG*TOK]
        for s in range(NSEG):
            out[b, s * 1024 + t * TOK:s * 1024 + (t + 1) * TOK, :] = \
                htc[:, s * TOK:(s + 1) * TOK].T
    return out, res


def kernel(**inputs):
    out, _ = _run(inputs, trace=False)
    return out



# revision 34
# speedup vs baseline: 1.7464x; 1.7464x over previous
"""RMT memory kernel for 8 Trainium2 NeuronCores (v2).

Core c = (batch b=c//4, token-quarter t=c%4); 256 tokens/segment/core.
Read-attention, gates, h token-local. Write-attention numerators/denoms
AllReduced per 4-core batch group (AR#1). Memory update replicated
post-AR; next-segment k_r/v_r projections K-sharded over the group and
summed in AR#2, which also redistributes the full memory state.

v2 structural changes vs v1:
- write-attention scores folded through host-precomputed
  AT[d, h*128+m] = sum_j q_w[m, hj] wk_w[d, hj]  (kills the per-segment
  wk_w gemm and its 13 MB stream)
- seg-0 k_r/v_r and q_w host-precomputed (no bootstrap AR)
- two-pass phase1 (q_r/gate) deprioritized so it fills AR windows
- weight streams batched via 3D-AP DMAs ([128, 5*512] chunks)
- psum evacuations on DVE; softmax denominators broadcast via
  ones-matmul; reciprocal_approx_fast instead of serial reciprocal
- gate gemm split: mem-half precomputed early, new-half post-AR#1
- x/h staged in fp16 (one DMA per segment each way)
"""

import sys

sys.path.insert(0, "/opt/trn_rl_repo")

import numpy as np
import ml_dtypes

D = 2560
H = 8
HD = 320
MEM = 128
NSEG = 4
TOK = 256
NT = D // 128       # 20
SL = 640            # per-core D slice
GROUPS = [[0, 1, 2, 3], [4, 5, 6, 7]]
SCALE = 1.0 / float(np.sqrt(HD))
BF16 = ml_dtypes.bfloat16

_CACHE = {}


def _pieces(start, size):
    """Split [start, start+size) at 128-boundaries."""
    out, p, end = [], start, start + size
    while p < end:
        nxt = min(end, (p // 128 + 1) * 128)
        out.append((p, nxt - p))
        p = nxt
    return out


def _build(debug=False, trunc=99):
    import concourse.bass as bass
    import concourse.bacc as bacc
    import concourse.mybir as mybir
    import concourse.tile as tile

    f32 = mybir.dt.float32
    f16 = mybir.dt.float16
    bf16 = mybir.dt.bfloat16
    AF = mybir.ActivationFunctionType

    nc = bacc.Bacc("TRN2", target_bir_lowering=False, debug=False, num_devices=8)

    xt16 = nc.dram_tensor("xt16", [D, NSEG * TOK], f16, kind="ExternalInput")
    xtb = nc.dram_tensor("xtb", [D, NSEG * TOK], bf16, kind="ExternalInput")
    wqr = nc.dram_tensor("wqr", [D, D], bf16, kind="ExternalInput")
    wgr = nc.dram_tensor("wgr", [D, D], bf16, kind="ExternalInput")
    wor = nc.dram_tensor("wor", [D, D], bf16, kind="ExternalInput")
    wvw = nc.dram_tensor("wvw", [D, D], f16, kind="ExternalInput")
    wow = nc.dram_tensor("wow", [D, D], bf16, kind="ExternalInput")
    at_d = nc.dram_tensor("at_d", [D, H * MEM], f16, kind="ExternalInput")
    wgw_sl = nc.dram_tensor("wgw_sl", [2 * D, SL], bf16, kind="ExternalInput")
    wkr_sl = nc.dram_tensor("wkr_sl", [SL, D], bf16, kind="ExternalInput")
    wvr_sl = nc.dram_tensor("wvr_sl", [SL, D], bf16, kind="ExternalInput")
    bgr = nc.dram_tensor("bgr", [D, 1], f32, kind="ExternalInput")
    bgw_sl = nc.dram_tensor("bgw_sl", [SL, 1], f32, kind="ExternalInput")
    m0t = nc.dram_tensor("m0t", [D, MEM], bf16, kind="ExternalInput")
    kr0 = nc.dram_tensor("kr0", [D, MEM], bf16, kind="ExternalInput")
    vr0 = nc.dram_tensor("vr0", [MEM, D], bf16, kind="ExternalInput")

    ht = nc.dram_tensor("ht", [D, NSEG * TOK], f16, kind="ExternalOutput")

    def r3(ap, a):
        """[ (a p), m ] DRAM slice -> [p, a, m] AP."""
        return ap.rearrange("(a p) m -> p a m", p=128)

    def t3(ap, a):
        """[p, a*m] SBUF tile AP -> [p, a, m]."""
        return ap.rearrange("p (a m) -> p a m", a=a)

    with tile.TileContext(nc) as tc:
        with (
            tc.tile_pool(name="dram", bufs=1, space="DRAM") as dram,
            tc.tile_pool(name="res", bufs=1) as res,
            tc.tile_pool(name="wrk", bufs=1) as wrk,
            tc.tile_pool(name="stream", bufs=1) as stream,
            tc.tile_pool(name="ps", bufs=8, space="PSUM") as psp,
        ):
            # ---------------- DRAM scratch ----------------
            # per-pass spill tiles (seg-major within pass) so segment s
            # only depends on its own pass's writes
            qr_sp = [dram.tile([128, 2 * NT * TOK], bf16, name=f"qr_sp{p}")
                     for p in range(2)]
            gt_sp = [dram.tile([128, 2 * NT * TOK], bf16, name=f"gt_sp{p}")
                     for p in range(2)]
            b1_in = dram.tile([129, NT * 128], bf16)
            b1_out = dram.tile([129, NT * 128], bf16)
            b2_in = dram.tile([384, NT * 128], bf16)
            b2_out = dram.tile([384, NT * 128], bf16)

            _psn = [0]

            def psum():
                _psn[0] += 1
                return psp.tile([128, 512], f32, tag="ps", bufs=8,
                                name=f"ps{_psn[0]}")

            _wln = [0]

            def wload(w, k0, nk, c0, ncols, tag="w", bufs=2, eng=None,
                      dt=None):
                _wln[0] += 1
                t = stream.tile([128, nk * ncols], dt or bf16, tag=tag,
                                bufs=bufs, name=f"wl{_wln[0]}")
                (eng or nc.sync).dma_start(
                    t3(t[:], nk),
                    r3(w[k0 * 128:(k0 + nk) * 128, c0:c0 + ncols], nk))
                return t

            # ---------------- residents ----------------
            mem = res.tile([128, NT * 128], bf16)
            krT = res.tile([128, NT * 128], bf16)
            vr = res.tile([128, NT * 128], bf16)
            memstage = res.tile([128, NT * 128], bf16)
            ones_bf = res.tile([128, 128], bf16)
            bgr_sb = res.tile([128, NT], f32)
            bgw_sb = res.tile([128, 5], f32)
            nc.vector.memset(ones_bf[:], 1.0)
            nc.vector.memset(memstage[:], 0.0)
            nc.sync.dma_start(t3(mem[:], NT), r3(m0t[:, :], NT))
            nc.sync.dma_start(t3(krT[:], NT), r3(kr0[:, :], NT))
            nc.sync.dma_start(vr[:], vr0[:, :])
            nc.sync.dma_start(t3(bgr_sb[:], NT), r3(bgr[:, :], NT))
            nc.sync.dma_start(t3(bgw_sb[:], 5), r3(bgw_sl[:, :], 5))

            pid = nc.vector.partition_id()
            toff = nc.snap((pid % 4) * SL, donate=False, min_val=0,
                           max_val=3 * SL)

            # ---------------- phase1 (q_r / gate), 2 passes ----------------
            def phase1_pass(p):
                xseg = stream.tile([128, NT * 512], bf16, tag="xseg", bufs=1)
                nc.sync.dma_start(
                    t3(xseg[:], NT),
                    r3(xtb[:, p * 512:(p + 1) * 512], NT))
                for w_dram, sp, is_gate in ((wqr, qr_sp[p], False),
                                            (wgr, gt_sp[p], True)):
                    for m0 in range(0, NT, 2):
                        pss = [psum() for _ in range(2)]
                        for k0 in range(0, NT, 5):
                            wt = wload(w_dram, k0, 5, m0 * 128, 256,
                                       tag="p1w", bufs=2)
                            for dk in range(5):
                                k = k0 + dk
                                for mi in range(2):
                                    nc.tensor.matmul(
                                        pss[mi],
                                        wt[:, dk * 256 + mi * 128:
                                           dk * 256 + (mi + 1) * 128],
                                        xseg[:, k * 512:(k + 1) * 512],
                                        start=(k == 0), stop=(k == NT - 1))
                        for mi in range(2):
                            m = m0 + mi
                            tmp = wrk.tile([128, 512], bf16, tag="p1ev",
                                           bufs=3, name=f"p1ev_{p}_{m}")
                            if is_gate:
                                nc.scalar.activation(
                                    tmp[:], pss[mi], AF.Sigmoid,
                                    bias=bgr_sb[:, m:m + 1])
                            else:
                                nc.vector.tensor_copy(tmp[:], pss[mi])
                            dst = sp[:].rearrange(
                                "p (s q) -> p s q", s=2)[
                                :, :, m * TOK:(m + 1) * TOK]
                            nc.gpsimd.dma_start(
                                dst, t3(tmp[:], 2))

            phase1_pass(0)
            phase1_pass(1)

            # ---------------- segment chain ----------------
            for s in range(NSEG):
                if 10 * s >= trunc:
                    break
                # -- per-segment loads --
                qrT_sb = wrk.tile([128, NT * TOK], bf16, tag="qrT")
                gtT_sb = wrk.tile([128, NT * TOK], bf16, tag="gtT")
                sh = s % 2
                nc.scalar.dma_start(
                    qrT_sb[:],
                    qr_sp[s // 2][:, sh * NT * TOK:(sh + 1) * NT * TOK])
                nc.scalar.dma_start(
                    gtT_sb[:],
                    gt_sp[s // 2][:, sh * NT * TOK:(sh + 1) * NT * TOK])
                h16 = wrk.tile([128, NT * TOK], f16, tag="h16")
                nc.scalar.dma_start(
                    t3(h16[:], NT), r3(xt16[:, s * TOK:(s + 1) * TOK], NT))
                if 10 * s + 1 >= trunc:
                    break

                # -- gate gemm, mem half (early; uses pre-update mem).
                #    Dead for the last segment (memory never read again).
                if s < NSEG - 1:
                    # one psum bank per m-block: a bank allows only one
                    # pending accumulation group (2KB zero region)
                    g_ps = [psum() for _ in range(5)]
                    for k0 in range(0, NT, 5):
                        wt = wload(wgw_sl, k0, 5, 0, SL)
                        for dk in range(5):
                            k = k0 + dk
                            for m in range(5):
                                nc.tensor.matmul(
                                    g_ps[m][:, 0:128],
                                    wt[:, dk * SL + m * 128:
                                       dk * SL + (m + 1) * 128],
                                    mem[:, k * 128:(k + 1) * 128],
                                    start=(k == 0), stop=(k == NT - 1))
                    gpre = wrk.tile([128, 5 * 128], f32, tag="gpre")
                    for m in range(5):
                        nc.vector.tensor_copy(
                            gpre[:, m * 128:(m + 1) * 128], g_ps[m][:, 0:128])

                if 10 * s + 5 >= trunc and trunc % 10 == 5:
                    break
                # -- read attention --
                E_sb = wrk.tile([128, H * TOK], bf16, tag="E")
                for h in range(H):
                    ps = psum()
                    pcs = _pieces(h * HD, HD)
                    for i, (off, sz) in enumerate(pcs):
                        g, o = off // 128, off % 128
                        nc.tensor.matmul(
                            ps[:, 0:TOK],
                            krT[o:o + sz, g * 128:(g + 1) * 128],
                            qrT_sb[o:o + sz, g * TOK:(g + 1) * TOK],
                            start=(i == 0), stop=(i == len(pcs) - 1))
                    nc.scalar.activation(
                        E_sb[:, h * TOK:(h + 1) * TOK],
                        ps[:, 0:TOK], AF.Exp, scale=SCALE)
                if 10 * s + 6 >= trunc and trunc % 10 == 6:
                    break
                # denominators -> broadcast reciprocal, one [128,512] tile
                # per head-pair (recip_approx_fast needs a contiguous dst)
                rrec = [wrk.tile([128, 512], f32, tag=f"rrec{hp}",
                                 name=f"rrec{hp}_{s}")
                        for hp in range(H // 2)]
                for hp in range(H // 2):
                    ps = psum()
                    nc.tensor.matmul(
                        ps[:], ones_bf[:],
                        E_sb[:, hp * 512:(hp + 1) * 512],
                        start=True, stop=True)
                    nc.vector.reciprocal_approx_fast(rrec[hp][:], ps[:])
                if 10 * s + 7 >= trunc and trunc % 10 == 7:
                    break
                # ctx (normalized at evacuation)
                ctx_sb = wrk.tile([128, NT * TOK], bf16, tag="ctx")
                ctx_ps = {}
                for h in range(H):
                    for off, sz in _pieces(h * HD, HD):
                        g, o = off // 128, off % 128
                        if g not in ctx_ps:
                            ctx_ps[g] = (psum(), [])
                        ps, evl = ctx_ps[g]
                        nc.tensor.matmul(
                            ps[o:o + sz, 0:TOK], vr[:, off:off + sz],
                            E_sb[:, h * TOK:(h + 1) * TOK],
                            start=True, stop=True)
                        evl.append((h, o, sz))
                        if o + sz == 128 or (h == H - 1 and off + sz == D):
                            for (hh, oo, ss) in evl:
                                nc.vector.tensor_mul(
                                    ctx_sb[oo:oo + ss, g * TOK:(g + 1) * TOK],
                                    ps[oo:oo + ss, 0:TOK],
                                    rrec[hh // 2][oo:oo + ss,
                                                  (hh % 2) * TOK:
                                                  (hh % 2 + 1) * TOK])
                            ctx_ps[g] = None

                if 10 * s + 8 >= trunc and trunc % 10 == 8:
                    break
                # -- o^T = wo_r.T @ ctx ; h = seg + gate*o (in-place f16) --
                for m0 in range(0, NT, 4):
                    pss = [psum() for _ in range(4)]
                    for k0 in range(0, NT, 5):
                        wt = wload(wor, k0, 5, m0 * 128, 512)
                        for dk in range(5):
                            k = k0 + dk
                            for mi in range(4):
                                nc.tensor.matmul(
                                    pss[mi][:, 0:TOK],
                                    wt[:, dk * 512 + mi * 128:
                                       dk * 512 + (mi + 1) * 128],
                                    ctx_sb[:, k * TOK:(k + 1) * TOK],
                                    start=(k == 0), stop=(k == NT - 1))
                    for mi in range(4):
                        m = m0 + mi
                        tmp32 = wrk.tile([128, TOK], f32, tag="otmp",
                                         bufs=2, name=f"otmp{s}_{m}")
                        nc.vector.tensor_mul(
                            tmp32[:], pss[mi][:, 0:TOK],
                            gtT_sb[:, m * TOK:(m + 1) * TOK])
                        nc.vector.tensor_add(
                            h16[:, m * TOK:(m + 1) * TOK], tmp32[:],
                            h16[:, m * TOK:(m + 1) * TOK])
                nc.scalar.dma_start(
                    r3(ht[:, s * TOK:(s + 1) * TOK], NT), t3(h16[:], NT))

                if 10 * s + 2 >= trunc:
                    break
                if s == NSEG - 1:
                    # the last segment's memory write is dead code:
                    # h(3) is the only output, memory is never read again
                    continue

                # -- v_w = h @ wv_w  [tok-part, 2*D] --
                vw = wrk.tile([128, 2 * D], bf16, tag="vw")
                for ch in range(5):
                    pss = [psum(), psum()]
                    for k0 in range(0, NT, 5):
                        wt = wload(wvw, k0, 5, ch * 512, 512, dt=f16)
                        for dk in range(5):
                            k = k0 + dk
                            for tt in range(2):
                                nc.tensor.matmul(
                                    pss[tt],
                                    h16[:, k * TOK + tt * 128:
                                        k * TOK + tt * 128 + 128],
                                    wt[:, dk * 512:(dk + 1) * 512],
                                    start=(k == 0), stop=(k == NT - 1))
                    for tt in range(2):
                        nc.vector.tensor_copy(
                            vw[:, tt * D + ch * 512:tt * D + (ch + 1) * 512],
                            pss[tt])

                # -- write scores via fold: ew^T[tok, mem] = h^T AT.
                #    One N=512 matmul covers 4 heads (shared h stationary);
                #    ewT column order: (tt, h, mem) --
                sc_ps = {}
                for tt in range(2):
                    for hg in range(2):
                        sc_ps[(tt, hg)] = psum()
                for k0 in range(0, NT, 2):
                    atw = wload(at_d, k0, 2, 0, H * 128, tag="atw", bufs=2,
                                dt=f16)
                    for dk in range(2):
                        k = k0 + dk
                        for tt in range(2):
                            for hg in range(2):
                                nc.tensor.matmul(
                                    sc_ps[(tt, hg)],
                                    h16[:, k * TOK + tt * 128:
                                        k * TOK + tt * 128 + 128],
                                    atw[:, dk * H * 128 + hg * 512:
                                        dk * H * 128 + (hg + 1) * 512],
                                    start=(k == 0), stop=(k == NT - 1))
                ewT = wrk.tile([128, H * 2 * 128], bf16, tag="ewT")

                def ewc(h, tt):  # ewT column offset for (head, token-half)
                    return tt * 1024 + h * 128

                for tt in range(2):
                    for hg in range(2):
                        nc.scalar.activation(
                            ewT[:, tt * 1024 + hg * 512:
                                tt * 1024 + (hg + 1) * 512],
                            sc_ps[(tt, hg)], AF.Exp, scale=SCALE)
                # write denominators (local partial, row-broadcast):
                # N=512 over 4 heads, accumulate the two token halves
                dbpack = wrk.tile([1, H * 128], bf16, tag="dbpack")
                for hg in range(2):
                    ps = psum()
                    for tt in range(2):
                        nc.tensor.matmul(
                            ps[:], ones_bf[:],
                            ewT[:, tt * 1024 + hg * 512:
                                tt * 1024 + (hg + 1) * 512],
                            start=(tt == 0), stop=(tt == 1))
                    nc.vector.tensor_copy(
                        dbpack[:, hg * 512:(hg + 1) * 512], ps[0:1, :])

                # -- attention numerators (partial over local tokens) --
                attn = wrk.tile([128, NT * 128], bf16, tag="attn")
                att_ps = {}
                for h in range(H):
                    for off, sz in _pieces(h * HD, HD):
                        g, o = off // 128, off % 128
                        if g not in att_ps:
                            att_ps[g] = (psum(), [])
                        ps, evl = att_ps[g]
                        for tt in range(2):
                            nc.tensor.matmul(
                                ps[o:o + sz, 0:128],
                                vw[:, tt * D + off:tt * D + off + sz],
                                ewT[:, ewc(h, tt):ewc(h, tt) + 128],
                                start=(tt == 0), stop=(tt == 1))
                        evl.append((h, o, sz))
                        if o + sz == 128 or (h == H - 1 and off + sz == D):
                            for (hh, oo, ss) in evl:
                                nc.vector.tensor_copy(
                                    attn[oo:oo + ss, g * 128:(g + 1) * 128],
                                    ps[oo:oo + ss, 0:128])
                            att_ps[g] = None

                # -- AR#1 --
                if 10 * s + 4 >= trunc:
                    break
                nc.scalar.dma_start(b1_in[0:128, :], attn[:])
                nc.scalar.dma_start(b1_in[128:129, 0:H * 128], dbpack[:])
                nc.gpsimd.collective_compute(
                    "AllReduce", mybir.AluOpType.add,
                    ins=[b1_in.opt()], outs=[b1_out.opt()],
                    replica_groups=GROUPS)
                nc.scalar.dma_start(attn[:], b1_out[0:128, :])
                nc.scalar.dma_start(dbpack[:], b1_out[128:129, 0:H * 128])
                dsum32 = wrk.tile([1, H * 128], f32, tag="dsum32")
                nc.vector.tensor_copy(dsum32[:], dbpack[:])
                drec = wrk.tile([1, H * 128], f32, tag="drec")
                nc.vector.reciprocal_approx_fast(drec[:], dsum32[:])
                rw = wrk.tile([128, H * 128], f32, tag="rw")
                nc.gpsimd.partition_broadcast(rw[:], drec[:])
                for h in range(H):
                    for off, sz in _pieces(h * HD, HD):
                        g, o = off // 128, off % 128
                        nc.vector.tensor_mul(
                            attn[o:o + sz, g * 128:(g + 1) * 128],
                            attn[o:o + sz, g * 128:(g + 1) * 128],
                            rw[o:o + sz, h * 128:(h + 1) * 128])

                # -- new_mem^T (full, replicated) --
                new_sb = wrk.tile([128, NT * 128], bf16, tag="new")
                for m0 in range(0, NT, 4):
                    pss = [psum() for _ in range(4)]
                    for k0 in range(0, NT, 5):
                        wt = wload(wow, k0, 5, m0 * 128, 512)
                        for dk in range(5):
                            k = k0 + dk
                            for mi in range(4):
                                nc.tensor.matmul(
                                    pss[mi][:, 0:128],
                                    wt[:, dk * 512 + mi * 128:
                                       dk * 512 + (mi + 1) * 128],
                                    attn[:, k * 128:(k + 1) * 128],
                                    start=(k == 0), stop=(k == NT - 1))
                    for mi in range(4):
                        nc.vector.tensor_copy(
                            new_sb[:, (m0 + mi) * 128:(m0 + mi + 1) * 128],
                            pss[mi][:, 0:128])

                # -- gate new half + sigmoid --
                g_ps2 = [psum() for _ in range(5)]
                for k0 in range(0, NT, 5):
                    wt = wload(wgw_sl, NT + k0, 5, 0, SL)
                    for dk in range(5):
                        k = k0 + dk
                        for m in range(5):
                            nc.tensor.matmul(
                                g_ps2[m][:, 0:128],
                                wt[:, dk * SL + m * 128:
                                   dk * SL + (m + 1) * 128],
                                new_sb[:, k * 128:(k + 1) * 128],
                                start=(k == 0), stop=(k == NT - 1))
                for m in range(5):
                    nc.vector.tensor_add(
                        gpre[:, m * 128:(m + 1) * 128],
                        g_ps2[m][:, 0:128],
                        gpre[:, m * 128:(m + 1) * 128])
                gsl = wrk.tile([128, 5 * 128], f32, tag="gsl")
                for m in range(5):
                    nc.scalar.activation(
                        gsl[:, m * 128:(m + 1) * 128],
                        gpre[:, m * 128:(m + 1) * 128],
                        AF.Sigmoid, bias=bgw_sb[:, m:m + 1])

                # -- memory update (slice-local) --
                dlt = wrk.tile([128, 5 * 128], f32, tag="dlt")
                nc.vector.tensor_sub(dlt[:], new_sb[:, bass.ds(toff, SL)],
                                     mem[:, bass.ds(toff, SL)])
                nc.vector.tensor_mul(dlt[:], gsl[:], dlt[:])
                memsl_n = wrk.tile([128, 5 * 128], bf16, tag="memsl_n")
                nc.vector.tensor_add(memsl_n[:], mem[:, bass.ds(toff, SL)],
                                     dlt[:])

                if 10 * s + 6 >= trunc:
                    break
                # -- k_r/v_r partials (K-sharded) + AR#2 --
                # pk: hold all 5 wkr row-block tiles, run the 20 m-block
                # groups sequentially (one pending group per bank slice)
                # HW: accumulation groups must start at column offset 0 of
                # a psum bank, so each m-block gets its own psum tile
                pkT = wrk.tile([128, NT * 128], bf16, tag="pkT")
                for half in range(2):
                    c0, ncol = half * (D // 2), D // 2
                    wkt = [wload(wkr_sl, kk, 1, c0, ncol, tag="wkv",
                                 bufs=5) for kk in range(5)]
                    for m in range(half * 10, half * 10 + 10):
                        ps = psum()
                        for kk in range(5):
                            nc.tensor.matmul(
                                ps[:, 0:128],
                                wkt[kk][:, (m - half * 10) * 128:
                                         (m - half * 10 + 1) * 128],
                                memsl_n[:, kk * 128:(kk + 1) * 128],
                                start=(kk == 0), stop=(kk == 4))
                        nc.vector.tensor_copy(
                            pkT[:, m * 128:(m + 1) * 128], ps[:, 0:128])
                pvv = wrk.tile([128, NT * 128], bf16, tag="pvv")
                for c0, nch in ((0, 2), (1024, 3)):
                    wvt = [wload(wvr_sl, kk, 1, c0, nch * 512, tag="wkv",
                                 bufs=5) for kk in range(5)]
                    for chl in range(nch):
                        ch = c0 // 512 + chl
                        ps = psum()
                        for kk in range(5):
                            nc.tensor.matmul(
                                ps, memsl_n[:, kk * 128:(kk + 1) * 128],
                                wvt[kk][:, chl * 512:(chl + 1) * 512],
                                start=(kk == 0), stop=(kk == 4))
                        nc.vector.tensor_copy(
                            pvv[:, ch * 512:(ch + 1) * 512], ps)

                nc.vector.tensor_copy(
                    memstage[:, bass.ds(toff, SL)], memsl_n[:])
                nc.scalar.dma_start(b2_in[0:128, :], pkT[:])
                nc.scalar.dma_start(b2_in[128:256, :], pvv[:])
                nc.scalar.dma_start(b2_in[256:384, :], memstage[:])
                nc.gpsimd.collective_compute(
                    "AllReduce", mybir.AluOpType.add,
                    ins=[b2_in.opt()], outs=[b2_out.opt()],
                    replica_groups=GROUPS)
                nc.scalar.dma_start(krT[:], b2_out[0:128, :])
                nc.scalar.dma_start(vr[:], b2_out[128:256, :])
                nc.scalar.dma_start(mem[:], b2_out[256:384, :])

    nc.compile()
    return nc


def _prep_inputs(inputs):
    hs = np.asarray(inputs["hidden_states"], np.float32)

    def bf(x):
        return np.ascontiguousarray(np.asarray(x, np.float32).astype(BF16))

    wq_w = np.asarray(inputs["wq_w"], np.float32)
    wk_w = np.asarray(inputs["wk_w"], np.float32)
    wq = np.asarray(inputs["write_queries"], np.float32)[0]
    m0 = np.asarray(inputs["initial_memory"], np.float32)[0]
    qw = wq @ wq_w  # [M, D]
    at = np.empty((D, H * MEM), np.float32)
    for h in range(H):
        hs_sl = slice(h * HD, (h + 1) * HD)
        at[:, h * MEM:(h + 1) * MEM] = wk_w[:, hs_sl] @ qw[:, hs_sl].T
    kr0 = (m0 @ np.asarray(inputs["wk_r"], np.float32)).T  # [D, M]
    vr0 = m0 @ np.asarray(inputs["wv_r"], np.float32)      # [M, D]

    shared = {
        "wqr": bf(inputs["wq_r"]), "wgr": bf(inputs["wg_r"]),
        "wor": bf(inputs["wo_r"]),
        "wvw": np.ascontiguousarray(
            np.asarray(inputs["wv_w"], np.float32).astype(np.float16)),
        "wow": bf(inputs["wo_w"]),
        "at_d": np.ascontiguousarray(at.astype(np.float16)),
        "kr0": bf(kr0), "vr0": bf(vr0),
        "bgr": np.ascontiguousarray(
            np.asarray(inputs["bg_r"], np.float32)[:, None]),
        "m0t": bf(m0.T),
    }
    wgw = np.asarray(inputs["wg_w"], np.float32)
    wkr = np.asarray(inputs["wk_r"], np.float32)
    wvr = np.asarray(inputs["wv_r"], np.float32)
    bgw = np.asarray(inputs["bg_w"], np.float32)

    in_maps = []
    for c in range(8):
        b, t = c // 4, c % 4
        cols = np.concatenate(
            [np.arange(s * 1024 + t * TOK, s * 1024 + (t + 1) * TOK)
             for s in range(NSEG)])
        xt = np.ascontiguousarray(hs[b, cols, :].T)  # [D, 1024]
        im = dict(shared)
        im["xt16"] = np.ascontiguousarray(xt.astype(np.float16))
        im["xtb"] = np.ascontiguousarray(xt.astype(BF16))
        im["wgw_sl"] = np.ascontiguousarray(
            wgw[:, t * SL:(t + 1) * SL].astype(BF16))
        im["wkr_sl"] = np.ascontiguousarray(
            wkr[t * SL:(t + 1) * SL, :].astype(BF16))
        im["wvr_sl"] = np.ascontiguousarray(
            wvr[t * SL:(t + 1) * SL, :].astype(BF16))
        im["bgw_sl"] = np.ascontiguousarray(bgw[t * SL:(t + 1) * SL, None])
        in_maps.append(im)
    return in_maps


def _run(inputs, trace=False, debug=False):
    import os
    from concourse.bass_utils import run_bass_kernel_spmd
    trunc = int(os.environ.get("KTRUNC", "99"))
    key = ("nc", debug, trunc)
    if key not in _CACHE:
        _CACHE[key] = _build(debug=debug, trunc=trunc)
    in_maps = _prep_inputs(inputs)
    res = run_bass_kernel_spmd(_CACHE[key], in_maps, list(range(8)),
                               trace=trace)
    hs = np.asarray(inputs["hidden_states"])
    out = np.empty((hs.shape[0], NSEG * 1024, D), np.float32)
    for c in range(8):
        b, t = c // 4, c % 4
        htc = np.asarray(res.results[c]["ht"], np.float32)  # [D, NSEG*TOK]
        for s in range(NSEG):
            out[b, s * 1024 + t * TOK:s * 1024 + (t + 1) * TOK, :] = \
                htc[:, s * TOK:(s + 1) * TOK].T
    return out, res


def kernel(**inputs):
    out, _ = _run(inputs, trace=False)
    return out


# revision 36
# speedup vs baseline: 1.8288x; 1.0472x over previous
"""RMT memory kernel for 8 Trainium2 NeuronCores (v2).

Core c = (batch b=c//4, token-quarter t=c%4); 256 tokens/segment/core.
Read-attention, gates, h token-local. Write-attention numerators/denoms
AllReduced per 4-core batch group (AR#1). Memory update replicated
post-AR; next-segment k_r/v_r projections K-sharded over the group and
summed in AR#2, which also redistributes the full memory state.

v2 structural changes vs v1:
- write-attention scores folded through host-precomputed
  AT[d, h*128+m] = sum_j q_w[m, hj] wk_w[d, hj]  (kills the per-segment
  wk_w gemm and its 13 MB stream)
- seg-0 k_r/v_r and q_w host-precomputed (no bootstrap AR)
- two-pass phase1 (q_r/gate) deprioritized so it fills AR windows
- weight streams batched via 3D-AP DMAs ([128, 5*512] chunks)
- psum evacuations on DVE; softmax denominators broadcast via
  ones-matmul; reciprocal_approx_fast instead of serial reciprocal
- gate gemm split: mem-half precomputed early, new-half post-AR#1
- x/h staged in fp16 (one DMA per segment each way)
"""

import sys

sys.path.insert(0, "/opt/trn_rl_repo")

import numpy as np
import ml_dtypes

D = 2560
H = 8
HD = 320
MEM = 128
NSEG = 4
TOK = 256
NT = D // 128       # 20
SL = 640            # per-core D slice
GROUPS = [[0, 1, 2, 3], [4, 5, 6, 7]]
SCALE = 1.0 / float(np.sqrt(HD))
BF16 = ml_dtypes.bfloat16

_CACHE = {}


def _pieces(start, size):
    """Split [start, start+size) at 128-boundaries."""
    out, p, end = [], start, start + size
    while p < end:
        nxt = min(end, (p // 128 + 1) * 128)
        out.append((p, nxt - p))
        p = nxt
    return out


def _build(debug=False, trunc=99,
           p1_waits=(0.0, 0.40, 0.72, 1.04)):
    import concourse.bass as bass
    import concourse.bacc as bacc
    import concourse.mybir as mybir
    import concourse.tile as tile

    f32 = mybir.dt.float32
    f16 = mybir.dt.float16
    bf16 = mybir.dt.bfloat16
    AF = mybir.ActivationFunctionType

    nc = bacc.Bacc("TRN2", target_bir_lowering=False, debug=False, num_devices=8)

    xt16 = nc.dram_tensor("xt16", [D, NSEG * TOK], f16, kind="ExternalInput")
    xtb = nc.dram_tensor("xtb", [D, NSEG * TOK], bf16, kind="ExternalInput")
    wqr = nc.dram_tensor("wqr", [D, D], bf16, kind="ExternalInput")
    wgr = nc.dram_tensor("wgr", [D, D], bf16, kind="ExternalInput")
    wor = nc.dram_tensor("wor", [D, D], bf16, kind="ExternalInput")
    wvw = nc.dram_tensor("wvw", [D, D], f16, kind="ExternalInput")
    wow = nc.dram_tensor("wow", [D, D], bf16, kind="ExternalInput")
    at_d = nc.dram_tensor("at_d", [D, H * MEM], f16, kind="ExternalInput")
    wgw_sl = nc.dram_tensor("wgw_sl", [2 * D, SL], bf16, kind="ExternalInput")
    wkr_sl = nc.dram_tensor("wkr_sl", [SL, D], bf16, kind="ExternalInput")
    wvr_sl = nc.dram_tensor("wvr_sl", [SL, D], bf16, kind="ExternalInput")
    bgr = nc.dram_tensor("bgr", [D, 1], f32, kind="ExternalInput")
    bgw_sl = nc.dram_tensor("bgw_sl", [SL, 1], f32, kind="ExternalInput")
    m0t = nc.dram_tensor("m0t", [D, MEM], bf16, kind="ExternalInput")
    kr0 = nc.dram_tensor("kr0", [D, MEM], bf16, kind="ExternalInput")
    vr0 = nc.dram_tensor("vr0", [MEM, D], bf16, kind="ExternalInput")

    ht = nc.dram_tensor("ht", [D, NSEG * TOK], f16, kind="ExternalOutput")

    def r3(ap, a):
        """[ (a p), m ] DRAM slice -> [p, a, m] AP."""
        return ap.rearrange("(a p) m -> p a m", p=128)

    def t3(ap, a):
        """[p, a*m] SBUF tile AP -> [p, a, m]."""
        return ap.rearrange("p (a m) -> p a m", a=a)

    P1_WAITS = p1_waits
    with tile.TileContext(nc) as tc:
        with (
            tc.tile_pool(name="dram", bufs=1, space="DRAM") as dram,
            tc.tile_pool(name="res", bufs=1) as res,
            tc.tile_pool(name="wrk", bufs=1) as wrk,
            tc.tile_pool(name="stream", bufs=1) as stream,
            tc.tile_pool(name="ps", bufs=8, space="PSUM") as psp,
        ):
            # ---------------- DRAM scratch ----------------
            # per-pass spill tiles (seg-major within pass) so segment s
            # only depends on its own pass's writes
            qr_sp = [dram.tile([128, 2 * NT * TOK], bf16, name=f"qr_sp{p}")
                     for p in range(2)]
            gt_sp = [dram.tile([128, 2 * NT * TOK], bf16, name=f"gt_sp{p}")
                     for p in range(2)]
            b1_in = dram.tile([129, NT * 128], bf16)
            b1_out = dram.tile([129, NT * 128], bf16)
            b2_in = dram.tile([384, NT * 128], bf16)
            b2_out = dram.tile([384, NT * 128], bf16)

            _psn = [0]

            def psum():
                _psn[0] += 1
                return psp.tile([128, 512], f32, tag="ps", bufs=8,
                                name=f"ps{_psn[0]}")

            _wln = [0]

            def wload(w, k0, nk, c0, ncols, tag="w", bufs=3, eng=None,
                      dt=None):
                _wln[0] += 1
                t = stream.tile([128, nk * ncols], dt or bf16, tag=tag,
                                bufs=bufs, name=f"wl{_wln[0]}")
                (eng or nc.sync).dma_start(
                    t3(t[:], nk),
                    r3(w[k0 * 128:(k0 + nk) * 128, c0:c0 + ncols], nk))
                return t

            # ---------------- residents ----------------
            mem = res.tile([128, NT * 128], bf16)
            krT = res.tile([128, NT * 128], bf16)
            vr = res.tile([128, NT * 128], bf16)
            memstage = res.tile([128, NT * 128], bf16)
            ones_bf = res.tile([128, 128], bf16)
            bgr_sb = res.tile([128, NT], f32)
            bgw_sb = res.tile([128, 5], f32)
            nc.vector.memset(ones_bf[:], 1.0)
            nc.vector.memset(memstage[:], 0.0)
            nc.sync.dma_start(t3(mem[:], NT), r3(m0t[:, :], NT))
            nc.sync.dma_start(t3(krT[:], NT), r3(kr0[:, :], NT))
            nc.sync.dma_start(vr[:], vr0[:, :])
            nc.sync.dma_start(t3(bgr_sb[:], NT), r3(bgr[:, :], NT))
            nc.sync.dma_start(t3(bgw_sb[:], 5), r3(bgw_sl[:, :], 5))

            pid = nc.vector.partition_id()
            toff = nc.snap((pid % 4) * SL, donate=False, min_val=0,
                           max_val=3 * SL)

            # ---------------- phase1 (q_r / gate), 4 per-seg units -------
            # Unit 0 runs immediately (segment 0 blocks on it); later units
            # carry tile_wait_until stamps so the scheduler parks them in
            # the AR windows instead of burning them on early stream stalls.
            def xseg_load(s):
                xseg = stream.tile([128, NT * TOK], bf16, tag="xseg",
                                   bufs=2, name=f"xseg{s}")
                nc.sync.dma_start(
                    t3(xseg[:], NT),
                    r3(xtb[:, s * TOK:(s + 1) * TOK], NT))
                return xseg

            def phase1_unit(s, xseg, which):
                sp = (qr_sp if which == "q" else gt_sp)[s // 2]
                w_dram = wqr if which == "q" else wgr
                sh = s % 2
                for m0 in range(0, NT, 2):
                    pss = [psum() for _ in range(2)]
                    for k0 in range(0, NT, 5):
                        wt = wload(w_dram, k0, 5, m0 * 128, 256,
                                   tag="p1w", bufs=3)
                        for dk in range(5):
                            k = k0 + dk
                            for mi in range(2):
                                nc.tensor.matmul(
                                    pss[mi][:, 0:TOK],
                                    wt[:, dk * 256 + mi * 128:
                                       dk * 256 + (mi + 1) * 128],
                                    xseg[:, k * TOK:(k + 1) * TOK],
                                    start=(k == 0), stop=(k == NT - 1))
                    for mi in range(2):
                        m = m0 + mi
                        tmp = wrk.tile([128, TOK], bf16, tag="p1ev",
                                       bufs=3, name=f"p1ev_{s}_{which}_{m}")
                        if which == "g":
                            nc.scalar.activation(
                                tmp[:], pss[mi][:, 0:TOK], AF.Sigmoid,
                                bias=bgr_sb[:, m:m + 1])
                        else:
                            nc.vector.tensor_copy(tmp[:], pss[mi][:, 0:TOK])
                        nc.gpsimd.dma_start(
                            sp[:, sh * NT * TOK + m * TOK:
                               sh * NT * TOK + (m + 1) * TOK], tmp[:])

            xs0 = xseg_load(0)
            phase1_unit(0, xs0, "q")
            phase1_unit(0, xs0, "g")
            for s_late in range(1, NSEG):
                xs = xseg_load(s_late)
                with tc.tile_wait_until(P1_WAITS[s_late]):
                    phase1_unit(s_late, xs, "q")
                    phase1_unit(s_late, xs, "g")

            # ---------------- segment chain ----------------
            for s in range(NSEG):
                if 10 * s >= trunc:
                    break
                # -- per-segment loads --
                qrT_sb = wrk.tile([128, NT * TOK], bf16, tag="qrT")
                gtT_sb = wrk.tile([128, NT * TOK], bf16, tag="gtT")
                sh = s % 2
                nc.scalar.dma_start(
                    qrT_sb[:],
                    qr_sp[s // 2][:, sh * NT * TOK:(sh + 1) * NT * TOK])
                nc.scalar.dma_start(
                    gtT_sb[:],
                    gt_sp[s // 2][:, sh * NT * TOK:(sh + 1) * NT * TOK])
                h16 = wrk.tile([128, NT * TOK], f16, tag="h16")
                nc.scalar.dma_start(
                    t3(h16[:], NT), r3(xt16[:, s * TOK:(s + 1) * TOK], NT))
                if 10 * s + 1 >= trunc:
                    break

                # -- gate gemm, mem half (early; uses pre-update mem).
                #    Dead for the last segment (memory never read again).
                if s < NSEG - 1:
                    # one psum bank per m-block: a bank allows only one
                    # pending accumulation group (2KB zero region)
                    g_ps = [psum() for _ in range(5)]
                    for k0 in range(0, NT, 5):
                        wt = wload(wgw_sl, k0, 5, 0, SL)
                        for dk in range(5):
                            k = k0 + dk
                            for m in range(5):
                                nc.tensor.matmul(
                                    g_ps[m][:, 0:128],
                                    wt[:, dk * SL + m * 128:
                                       dk * SL + (m + 1) * 128],
                                    mem[:, k * 128:(k + 1) * 128],
                                    start=(k == 0), stop=(k == NT - 1))
                    gpre = wrk.tile([128, 5 * 128], f32, tag="gpre")
                    for m in range(5):
                        nc.vector.tensor_copy(
                            gpre[:, m * 128:(m + 1) * 128], g_ps[m][:, 0:128])

                if 10 * s + 5 >= trunc and trunc % 10 == 5:
                    break
                # -- read attention --
                E_sb = wrk.tile([128, H * TOK], bf16, tag="E")
                for h in range(H):
                    ps = psum()
                    pcs = _pieces(h * HD, HD)
                    for i, (off, sz) in enumerate(pcs):
                        g, o = off // 128, off % 128
                        nc.tensor.matmul(
                            ps[:, 0:TOK],
                            krT[o:o + sz, g * 128:(g + 1) * 128],
                            qrT_sb[o:o + sz, g * TOK:(g + 1) * TOK],
                            start=(i == 0), stop=(i == len(pcs) - 1))
                    nc.scalar.activation(
                        E_sb[:, h * TOK:(h + 1) * TOK],
                        ps[:, 0:TOK], AF.Exp, scale=SCALE)
                if 10 * s + 6 >= trunc and trunc % 10 == 6:
                    break
                # denominators -> broadcast reciprocal, one [128,512] tile
                # per head-pair (recip_approx_fast needs a contiguous dst)
                rrec = [wrk.tile([128, 512], f32, tag=f"rrec{hp}",
                                 name=f"rrec{hp}_{s}")
                        for hp in range(H // 2)]
                for hp in range(H // 2):
                    ps = psum()
                    nc.tensor.matmul(
                        ps[:], ones_bf[:],
                        E_sb[:, hp * 512:(hp + 1) * 512],
                        start=True, stop=True)
                    nc.vector.reciprocal_approx_fast(rrec[hp][:], ps[:])
                if 10 * s + 7 >= trunc and trunc % 10 == 7:
                    break
                # ctx (normalized at evacuation)
                ctx_sb = wrk.tile([128, NT * TOK], bf16, tag="ctx")
                ctx_ps = {}
                for h in range(H):
                    for off, sz in _pieces(h * HD, HD):
                        g, o = off // 128, off % 128
                        if g not in ctx_ps:
                            ctx_ps[g] = (psum(), [])
                        ps, evl = ctx_ps[g]
                        nc.tensor.matmul(
                            ps[o:o + sz, 0:TOK], vr[:, off:off + sz],
                            E_sb[:, h * TOK:(h + 1) * TOK],
                            start=True, stop=True)
                        evl.append((h, o, sz))
                        if o + sz == 128 or (h == H - 1 and off + sz == D):
                            for (hh, oo, ss) in evl:
                                nc.vector.tensor_mul(
                                    ctx_sb[oo:oo + ss, g * TOK:(g + 1) * TOK],
                                    ps[oo:oo + ss, 0:TOK],
                                    rrec[hh // 2][oo:oo + ss,
                                                  (hh % 2) * TOK:
                                                  (hh % 2 + 1) * TOK])
                            ctx_ps[g] = None

                if 10 * s + 8 >= trunc and trunc % 10 == 8:
                    break
                # -- o^T = wo_r.T @ ctx ; h = seg + gate*o (in-place f16) --
                for m0 in range(0, NT, 4):
                    pss = [psum() for _ in range(4)]
                    for k0 in range(0, NT, 5):
                        wt = wload(wor, k0, 5, m0 * 128, 512)
                        for dk in range(5):
                            k = k0 + dk
                            for mi in range(4):
                                nc.tensor.matmul(
                                    pss[mi][:, 0:TOK],
                                    wt[:, dk * 512 + mi * 128:
                                       dk * 512 + (mi + 1) * 128],
                                    ctx_sb[:, k * TOK:(k + 1) * TOK],
                                    start=(k == 0), stop=(k == NT - 1))
                    for mi in range(4):
                        m = m0 + mi
                        tmp32 = wrk.tile([128, TOK], f32, tag="otmp",
                                         bufs=2, name=f"otmp{s}_{m}")
                        nc.vector.tensor_mul(
                            tmp32[:], pss[mi][:, 0:TOK],
                            gtT_sb[:, m * TOK:(m + 1) * TOK])
                        nc.vector.tensor_add(
                            h16[:, m * TOK:(m + 1) * TOK], tmp32[:],
                            h16[:, m * TOK:(m + 1) * TOK])
                nc.scalar.dma_start(
                    r3(ht[:, s * TOK:(s + 1) * TOK], NT), t3(h16[:], NT))

                if 10 * s + 2 >= trunc:
                    break
                if s == NSEG - 1:
                    # the last segment's memory write is dead code:
                    # h(3) is the only output, memory is never read again
                    continue

                # -- v_w = h @ wv_w  [tok-part, 2*D] --
                vw = wrk.tile([128, 2 * D], bf16, tag="vw")
                for ch in range(5):
                    pss = [psum(), psum()]
                    for k0 in range(0, NT, 5):
                        wt = wload(wvw, k0, 5, ch * 512, 512, dt=f16)
                        for dk in range(5):
                            k = k0 + dk
                            for tt in range(2):
                                nc.tensor.matmul(
                                    pss[tt],
                                    h16[:, k * TOK + tt * 128:
                                        k * TOK + tt * 128 + 128],
                                    wt[:, dk * 512:(dk + 1) * 512],
                                    start=(k == 0), stop=(k == NT - 1))
                    for tt in range(2):
                        nc.vector.tensor_copy(
                            vw[:, tt * D + ch * 512:tt * D + (ch + 1) * 512],
                            pss[tt])

                # -- write scores via fold: ew^T[tok, mem] = h^T AT.
                #    One N=512 matmul covers 4 heads (shared h stationary);
                #    ewT column order: (tt, h, mem) --
                sc_ps = {}
                for tt in range(2):
                    for hg in range(2):
                        sc_ps[(tt, hg)] = psum()
                for k0 in range(0, NT, 2):
                    atw = wload(at_d, k0, 2, 0, H * 128, tag="atw", bufs=2,
                                dt=f16)
                    for dk in range(2):
                        k = k0 + dk
                        for tt in range(2):
                            for hg in range(2):
                                nc.tensor.matmul(
                                    sc_ps[(tt, hg)],
                                    h16[:, k * TOK + tt * 128:
                                        k * TOK + tt * 128 + 128],
                                    atw[:, dk * H * 128 + hg * 512:
                                        dk * H * 128 + (hg + 1) * 512],
                                    start=(k == 0), stop=(k == NT - 1))
                ewT = wrk.tile([128, H * 2 * 128], bf16, tag="ewT")

                def ewc(h, tt):  # ewT column offset for (head, token-half)
                    return tt * 1024 + h * 128

                for tt in range(2):
                    for hg in range(2):
                        nc.scalar.activation(
                            ewT[:, tt * 1024 + hg * 512:
                                tt * 1024 + (hg + 1) * 512],
                            sc_ps[(tt, hg)], AF.Exp, scale=SCALE)
                # write denominators (local partial, row-broadcast):
                # N=512 over 4 heads, accumulate the two token halves
                dbpack = wrk.tile([1, H * 128], bf16, tag="dbpack")
                for hg in range(2):
                    ps = psum()
                    for tt in range(2):
                        nc.tensor.matmul(
                            ps[:], ones_bf[:],
                            ewT[:, tt * 1024 + hg * 512:
                                tt * 1024 + (hg + 1) * 512],
                            start=(tt == 0), stop=(tt == 1))
                    nc.vector.tensor_copy(
                        dbpack[:, hg * 512:(hg + 1) * 512], ps[0:1, :])

                # -- attention numerators (partial over local tokens) --
                attn = wrk.tile([128, NT * 128], bf16, tag="attn")
                att_ps = {}
                for h in range(H):
                    for off, sz in _pieces(h * HD, HD):
                        g, o = off // 128, off % 128
                        if g not in att_ps:
                            att_ps[g] = (psum(), [])
                        ps, evl = att_ps[g]
                        for tt in range(2):
                            nc.tensor.matmul(
                                ps[o:o + sz, 0:128],
                                vw[:, tt * D + off:tt * D + off + sz],
                                ewT[:, ewc(h, tt):ewc(h, tt) + 128],
                                start=(tt == 0), stop=(tt == 1))
                        evl.append((h, o, sz))
                        if o + sz == 128 or (h == H - 1 and off + sz == D):
                            for (hh, oo, ss) in evl:
                                nc.vector.tensor_copy(
                                    attn[oo:oo + ss, g * 128:(g + 1) * 128],
                                    ps[oo:oo + ss, 0:128])
                            att_ps[g] = None

                # -- AR#1 --
                if 10 * s + 4 >= trunc:
                    break
                nc.scalar.dma_start(b1_in[0:128, :], attn[:])
                nc.scalar.dma_start(b1_in[128:129, 0:H * 128], dbpack[:])
                nc.gpsimd.collective_compute(
                    "AllReduce", mybir.AluOpType.add,
                    ins=[b1_in.opt()], outs=[b1_out.opt()],
                    replica_groups=GROUPS)
                nc.scalar.dma_start(attn[:], b1_out[0:128, :])
                nc.scalar.dma_start(dbpack[:], b1_out[128:129, 0:H * 128])
                dsum32 = wrk.tile([1, H * 128], f32, tag="dsum32")
                nc.vector.tensor_copy(dsum32[:], dbpack[:])
                drec = wrk.tile([1, H * 128], f32, tag="drec")
                nc.vector.reciprocal_approx_fast(drec[:], dsum32[:])
                rw = wrk.tile([128, H * 128], f32, tag="rw")
                nc.gpsimd.partition_broadcast(rw[:], drec[:])
                for h in range(H):
                    for off, sz in _pieces(h * HD, HD):
                        g, o = off // 128, off % 128
                        nc.vector.tensor_mul(
                            attn[o:o + sz, g * 128:(g + 1) * 128],
                            attn[o:o + sz, g * 128:(g + 1) * 128],
                            rw[o:o + sz, h * 128:(h + 1) * 128])

                # -- new_mem^T (full, replicated) --
                new_sb = wrk.tile([128, NT * 128], bf16, tag="new")
                for m0 in range(0, NT, 4):
                    pss = [psum() for _ in range(4)]
                    for k0 in range(0, NT, 5):
                        wt = wload(wow, k0, 5, m0 * 128, 512)
                        for dk in range(5):
                            k = k0 + dk
                            for mi in range(4):
                                nc.tensor.matmul(
                                    pss[mi][:, 0:128],
                                    wt[:, dk * 512 + mi * 128:
                                       dk * 512 + (mi + 1) * 128],
                                    attn[:, k * 128:(k + 1) * 128],
                                    start=(k == 0), stop=(k == NT - 1))
                    for mi in range(4):
                        nc.vector.tensor_copy(
                            new_sb[:, (m0 + mi) * 128:(m0 + mi + 1) * 128],
                            pss[mi][:, 0:128])

                # -- gate new half + sigmoid --
                g_ps2 = [psum() for _ in range(5)]
                for k0 in range(0, NT, 5):
                    wt = wload(wgw_sl, NT + k0, 5, 0, SL)
                    for dk in range(5):
                        k = k0 + dk
                        for m in range(5):
                            nc.tensor.matmul(
                                g_ps2[m][:, 0:128],
                                wt[:, dk * SL + m * 128:
                                   dk * SL + (m + 1) * 128],
                                new_sb[:, k * 128:(k + 1) * 128],
                                start=(k == 0), stop=(k == NT - 1))
                for m in range(5):
                    nc.vector.tensor_add(
                        gpre[:, m * 128:(m + 1) * 128],
                        g_ps2[m][:, 0:128],
                        gpre[:, m * 128:(m + 1) * 128])
                gsl = wrk.tile([128, 5 * 128], f32, tag="gsl")
                for m in range(5):
                    nc.scalar.activation(
                        gsl[:, m * 128:(m + 1) * 128],
                        gpre[:, m * 128:(m + 1) * 128],
                        AF.Sigmoid, bias=bgw_sb[:, m:m + 1])

                # -- memory update (slice-local) --
                dlt = wrk.tile([128, 5 * 128], f32, tag="dlt")
                nc.vector.tensor_sub(dlt[:], new_sb[:, bass.ds(toff, SL)],
                                     mem[:, bass.ds(toff, SL)])
                nc.vector.tensor_mul(dlt[:], gsl[:], dlt[:])
                memsl_n = wrk.tile([128, 5 * 128], bf16, tag="memsl_n")
                nc.vector.tensor_add(memsl_n[:], mem[:, bass.ds(toff, SL)],
                                     dlt[:])

                if 10 * s + 6 >= trunc:
                    break
                # -- k_r/v_r partials (K-sharded) + AR#2 --
                # pk: hold all 5 wkr row-block tiles, run the 20 m-block
                # groups sequentially (one pending group per bank slice)
                # HW: accumulation groups must start at column offset 0 of
                # a psum bank, so each m-block gets its own psum tile
                pkT = wrk.tile([128, NT * 128], bf16, tag="pkT")
                for half in range(2):
                    c0, ncol = half * (D // 2), D // 2
                    wkt = [wload(wkr_sl, kk, 1, c0, ncol, tag="wkv",
                                 bufs=5) for kk in range(5)]
                    for m in range(half * 10, half * 10 + 10):
                        ps = psum()
                        for kk in range(5):
                            nc.tensor.matmul(
                                ps[:, 0:128],
                                wkt[kk][:, (m - half * 10) * 128:
                                         (m - half * 10 + 1) * 128],
                                memsl_n[:, kk * 128:(kk + 1) * 128],
                                start=(kk == 0), stop=(kk == 4))
                        nc.vector.tensor_copy(
                            pkT[:, m * 128:(m + 1) * 128], ps[:, 0:128])
                pvv = wrk.tile([128, NT * 128], bf16, tag="pvv")
                for c0, nch in ((0, 2), (1024, 3)):
                    wvt = [wload(wvr_sl, kk, 1, c0, nch * 512, tag="wkv",
                                 bufs=5) for kk in range(5)]
                    for chl in range(nch):
                        ch = c0 // 512 + chl
                        ps = psum()
                        for kk in range(5):
                            nc.tensor.matmul(
                                ps, memsl_n[:, kk * 128:(kk + 1) * 128],
                                wvt[kk][:, chl * 512:(chl + 1) * 512],
                                start=(kk == 0), stop=(kk == 4))
                        nc.vector.tensor_copy(
                            pvv[:, ch * 512:(ch + 1) * 512], ps)

                nc.vector.tensor_copy(
                    memstage[:, bass.ds(toff, SL)], memsl_n[:])
                nc.scalar.dma_start(b2_in[0:128, :], pkT[:])
                nc.scalar.dma_start(b2_in[128:256, :], pvv[:])
                nc.scalar.dma_start(b2_in[256:384, :], memstage[:])
                nc.gpsimd.collective_compute(
                    "AllReduce", mybir.AluOpType.add,
                    ins=[b2_in.opt()], outs=[b2_out.opt()],
                    replica_groups=GROUPS)
                nc.scalar.dma_start(krT[:], b2_out[0:128, :])
                nc.scalar.dma_start(vr[:], b2_out[128:256, :])
                nc.scalar.dma_start(mem[:], b2_out[256:384, :])

    nc.compile()
    return nc


def _prep_inputs(inputs):
    hs = np.asarray(inputs["hidden_states"], np.float32)

    def bf(x):
        return np.ascontiguousarray(np.asarray(x, np.float32).astype(BF16))

    wq_w = np.asarray(inputs["wq_w"], np.float32)
    wk_w = np.asarray(inputs["wk_w"], np.float32)
    wq = np.asarray(inputs["write_queries"], np.float32)[0]
    m0 = np.asarray(inputs["initial_memory"], np.float32)[0]
    qw = wq @ wq_w  # [M, D]
    at = np.empty((D, H * MEM), np.float32)
    for h in range(H):
        hs_sl = slice(h * HD, (h + 1) * HD)
        at[:, h * MEM:(h + 1) * MEM] = wk_w[:, hs_sl] @ qw[:, hs_sl].T
    kr0 = (m0 @ np.asarray(inputs["wk_r"], np.float32)).T  # [D, M]
    vr0 = m0 @ np.asarray(inputs["wv_r"], np.float32)      # [M, D]

    shared = {
        "wqr": bf(inputs["wq_r"]), "wgr": bf(inputs["wg_r"]),
        "wor": bf(inputs["wo_r"]),
        "wvw": np.ascontiguousarray(
            np.asarray(inputs["wv_w"], np.float32).astype(np.float16)),
        "wow": bf(inputs["wo_w"]),
        "at_d": np.ascontiguousarray(at.astype(np.float16)),
        "kr0": bf(kr0), "vr0": bf(vr0),
        "bgr": np.ascontiguousarray(
            np.asarray(inputs["bg_r"], np.float32)[:, None]),
        "m0t": bf(m0.T),
    }
    wgw = np.asarray(inputs["wg_w"], np.float32)
    wkr = np.asarray(inputs["wk_r"], np.float32)
    wvr = np.asarray(inputs["wv_r"], np.float32)
    bgw = np.asarray(inputs["bg_w"], np.float32)

    in_maps = []
    for c in range(8):
        b, t = c // 4, c % 4
        cols = np.concatenate(
            [np.arange(s * 1024 + t * TOK, s * 1024 + (t + 1) * TOK)
             for s in range(NSEG)])
        xt = np.ascontiguousarray(hs[b, cols, :].T)  # [D, 1024]
        im = dict(shared)
        im["xt16"] = np.ascontiguousarray(xt.astype(np.float16))
        im["xtb"] = np.ascontiguousarray(xt.astype(BF16))
        im["wgw_sl"] = np.ascontiguousarray(
            wgw[:, t * SL:(t + 1) * SL].astype(BF16))
        im["wkr_sl"] = np.ascontiguousarray(
            wkr[t * SL:(t + 1) * SL, :].astype(BF16))
        im["wvr_sl"] = np.ascontiguousarray(
            wvr[t * SL:(t + 1) * SL, :].astype(BF16))
        im["bgw_sl"] = np.ascontiguousarray(bgw[t * SL:(t + 1) * SL, None])
        in_maps.append(im)
    return in_maps


def _run(inputs, trace=False, debug=False):
    import os
    from concourse.bass_utils import run_bass_kernel_spmd
    trunc = int(os.environ.get("KTRUNC", "99"))
    key = ("nc", debug, trunc)
    if key not in _CACHE:
        _CACHE[key] = _build(debug=debug, trunc=trunc)
    in_maps = _prep_inputs(inputs)
    res = run_bass_kernel_spmd(_CACHE[key], in_maps, list(range(8)),
                               trace=trace)
    hs = np.asarray(inputs["hidden_states"])
    out = np.empty((hs.shape[0], NSEG * 1024, D), np.float32)
    for c in range(8):
        b, t = c // 4, c % 4
        htc = np.asarray(res.results[c]["ht"], np.float32)  # [D, NSEG*TOK]
        for s in range(NSEG):
            out[b, s * 1024 + t * TOK:s * 1024 + (t + 1) * TOK, :] = \
                htc[:, s * TOK:(s + 1) * TOK].T
    return out, res


def kernel(**inputs):
    out, _ = _run(inputs, trace=False)
    return out
